# revision 1
# baseline (speedup 1.0000x reference)
"""Trainium2 Bass kernel for nn_DynamicKernelSelection (moe_routing).

Strategy
--------
Host side (cheap, O(B*C) work):
  * Gating: mean-pool + tiny matmul + argmax in float64 (margins for the
    reference inputs are >=1.3e-5, far above fp32 noise, so the fp64 argmax
    agrees with the reference's fp32 argmax).
  * Expert selection: pick the selected depthwise kernel per sample; a
    smaller kernel center-embedded in a larger one (same dilation) yields an
    identical "same"-padded conv, so stage-1 kernels are padded to 5x5 and
    stage-2 kernels to 11x11 -> one uniform SPMD device program.
  * Build banded Toeplitz lhsT matrices per (sample, channel, kernel
    column): T[h', h] = W[c, dh, dw] at h' = h + (dh-c0)*dil.  The H
    (row) taps of the depthwise conv then become a single 128x128 fp32
    matmul per kernel column, and the W (column) shifts are realized by
    accumulating the k_w matmuls into PSUM at shifted output column
    ranges (zero padding falls out of the clipped band / clipped ranges).

Device side (per core = 2 samples, data-parallel over batch):
  per sample, per channel c:
    stage1: 5 matmuls  (lhsT = T1[c,dw], rhs = x[c] as [H,W] tile) -> PSUM
            evac + bias on VectorE -> out1 tile (SBUF, also DMAed out)
    stage2: 11 matmuls (lhsT = T2[c,dw], rhs = out1[c]) -> PSUM
            evac + bias -> out2 chunk -> DMA out
All tensors are pre-laid-out on the host so every DMA is a contiguous
[128-partition x contiguous-free] transfer.
"""

import numpy as np

B, C, H, W = 16, 128, 128, 128
N_CORES = 8
SPC = B // N_CORES  # samples per core
K1, K2 = 5, 11      # uniform (padded) kernel sizes
DIL1, DIL2 = 1, 3
C0_1, C0_2 = K1 // 2, K2 // 2

_PROG = None  # compiled bass program, cached across calls


def _gating(x, aw1, ab1, aw2, ab2):
    pooled = x.astype(np.float64).mean(axis=(2, 3))          # [B, C]
    l1 = pooled @ aw1.astype(np.float64).T + ab1.astype(np.float64)
    l2 = pooled @ aw2.astype(np.float64).T + ab2.astype(np.float64)
    return l1.argmax(axis=1), l2.argmax(axis=1)


def _pad_kernel(w, k_to):
    # w: [C, 1, k, k] -> [C, k_to, k_to] center-embedded
    w = np.ascontiguousarray(w[:, 0]).astype(np.float32)
    k = w.shape[-1]
    p = (k_to - k) // 2
    return np.pad(w, ((0, 0), (p, p), (p, p)))


def _band(wk, k, dil):
    """wk: [C, k, k] -> banded lhsT stack [C, H(h'), k(dw), H(h)] fp32."""
    t = np.zeros((C, H, k, H), np.float32)
    tv = t.transpose(1, 3, 0, 2)  # [h', h, C, dw] view
    c0 = k // 2
    for dh in range(k):
        d = (dh - c0) * dil
        h = np.arange(max(0, -d), H - max(0, d))
        tv[h + d, h] = wk[:, dh, :]
    return t


def _build_program():
    import concourse.tile as tile
    from concourse import bacc, mybir

    dt = mybir.dt.float32
    nc = bacc.Bacc("TRN2", target_bir_lowering=False, debug=False,
                   enable_asserts=False, num_devices=N_CORES)

    x2 = nc.dram_tensor("x2", [SPC, H, C * W], dt, kind="ExternalInput").ap()
    t1 = nc.dram_tensor("t1", [SPC, C, H, K1 * H], dt, kind="ExternalInput").ap()
    t2 = nc.dram_tensor("t2", [SPC, C, H, K2 * H], dt, kind="ExternalInput").ap()
    b1r = nc.dram_tensor("b1r", [SPC, H, C], dt, kind="ExternalInput").ap()
    b2r = nc.dram_tensor("b2r", [SPC, H, C], dt, kind="ExternalInput").ap()
    o1d = nc.dram_tensor("o1", [SPC, H, C * W], dt, kind="ExternalOutput").ap()
    o2d = nc.dram_tensor("o2", [SPC, H, C * W], dt, kind="ExternalOutput").ap()

    CHUNK = 16  # channels per out2 staging chunk
    add = mybir.AluOpType.add

    with tile.TileContext(nc) as tc:
        with (tc.tile_pool(name="xp", bufs=1) as xp,
              tc.tile_pool(name="o1p", bufs=1) as o1p,
              tc.tile_pool(name="o2p", bufs=2) as o2p,
              tc.tile_pool(name="t1p", bufs=3) as t1p,
              tc.tile_pool(name="t2p", bufs=3) as t2p,
              tc.tile_pool(name="bp", bufs=2) as bp,
              tc.tile_pool(name="ps", bufs=6, space="PSUM") as ps):
            for s in range(SPC):
                xs = xp.tile([128, C * W], dt, tag="x")
                nc.sync.dma_start(out=xs[:], in_=x2[s])
                b1t = bp.tile([128, C], dt, tag="b1")
                nc.sync.dma_start(out=b1t[:], in_=b1r[s])
                b2t = bp.tile([128, C], dt, tag="b2")
                nc.sync.dma_start(out=b2t[:], in_=b2r[s])
                o1s = o1p.tile([128, C * W], dt, tag="o1")

                order1 = [C0_1] + [dw for dw in range(K1) if dw != C0_1]
                order2 = [C0_2] + [dw for dw in range(K2) if dw != C0_2]

                # stage 1
                for c in range(C):
                    t1t = t1p.tile([128, K1 * H], dt, tag="t1")
                    nc.sync.dma_start(out=t1t[:], in_=t1[s, c])
                    p1 = ps.tile([128, W], dt, tag="ps")
                    for j, dw in enumerate(order1):
                        d = (dw - C0_1) * DIL1
                        a = max(0, -d)
                        ln = W - abs(d)
                        nc.tensor.matmul(
                            out=p1[:, a:a + ln],
                            lhsT=t1t[:, dw * H:(dw + 1) * H],
                            rhs=xs[:, c * W + a + d: c * W + a + d + ln],
                            start=(j == 0), stop=(j == len(order1) - 1),
                            skip_group_check=True)
                    nc.vector.tensor_scalar(
                        out=o1s[:, c * W:(c + 1) * W], in0=p1[:],
                        scalar1=b1t[:, c:c + 1], scalar2=None, op0=add)
                nc.sync.dma_start(out=o1d[s], in_=o1s[:])

                # stage 2
                for cg in range(0, C, CHUNK):
                    o2c = o2p.tile([128, CHUNK * W], dt, tag="o2")
                    for ci in range(CHUNK):
                        c = cg + ci
                        t2t = t2p.tile([128, K2 * H], dt, tag="t2")
                        nc.sync.dma_start(out=t2t[:], in_=t2[s, c])
                        p2 = ps.tile([128, W], dt, tag="ps")
                        for j, dw in enumerate(order2):
                            d = (dw - C0_2) * DIL2
                            a = max(0, -d)
                            ln = W - abs(d)
                            nc.tensor.matmul(
                                out=p2[:, a:a + ln],
                                lhsT=t2t[:, dw * H:(dw + 1) * H],
                                rhs=o1s[:, c * W + a + d: c * W + a + d + ln],
                                start=(j == 0), stop=(j == len(order2) - 1),
                                skip_group_check=True)
                        nc.vector.tensor_scalar(
                            out=o2c[:, ci * W:(ci + 1) * W], in0=p2[:],
                            scalar1=b2t[:, c:c + 1], scalar2=None, op0=add)
                    nc.sync.dma_start(out=o2d[s, :, cg * W:(cg + CHUNK) * W],
                                      in_=o2c[:])
    nc.compile()
    return nc


def kernel(x, aw1, ab1, aw2, ab2, w1_3, b1_3, w1_5, b1_5,
           w2_7, b2_7, w2_9, b2_9, w2_11, b2_11):
    global _PROG
    from concourse.bass_utils import run_bass_kernel_spmd

    x = np.ascontiguousarray(np.asarray(x, dtype=np.float32))
    assert x.shape == (B, C, H, W)

    idx1, idx2 = _gating(np.asarray(x), np.asarray(aw1), np.asarray(ab1),
                         np.asarray(aw2), np.asarray(ab2))

    # banded lhsT stacks per expert (built once per call; [C, H, k, H])
    w1e = [_pad_kernel(np.asarray(w1_3), K1), _pad_kernel(np.asarray(w1_5), K1)]
    w2e = [_pad_kernel(np.asarray(w2_7), K2), _pad_kernel(np.asarray(w2_9), K2),
           _pad_kernel(np.asarray(w2_11), K2)]
    t1e = [np.ascontiguousarray(_band(w, K1, DIL1).reshape(C, H, K1 * H))
           for w in w1e]
    t2e = [np.ascontiguousarray(_band(w, K2, DIL2).reshape(C, H, K2 * H))
           for w in w2e]
    b1e = [np.asarray(b1_3, np.float32), np.asarray(b1_5, np.float32)]
    b2e = [np.asarray(b2_7, np.float32), np.asarray(b2_9, np.float32),
           np.asarray(b2_11, np.float32)]

    if _PROG is None:
        _PROG = _build_program()
    nc = _PROG

    # x in [s, h, c, w] layout so each DMA partition row is contiguous
    xt = np.ascontiguousarray(x.transpose(0, 2, 1, 3)).reshape(B, H, C * W)

    in_maps = []
    pairs = [tuple(range(i * SPC, (i + 1) * SPC)) for i in range(N_CORES)]
    for pair in pairs:
        m = {
            "x2": np.ascontiguousarray(xt[list(pair)]),
            "t1": np.stack([t1e[idx1[s]] for s in pair]),
            "t2": np.stack([t2e[idx2[s]] for s in pair]),
            "b1r": np.ascontiguousarray(np.broadcast_to(
                np.stack([b1e[idx1[s]] for s in pair])[:, None, :],
                (SPC, H, C))),
            "b2r": np.ascontiguousarray(np.broadcast_to(
                np.stack([b2e[idx2[s]] for s in pair])[:, None, :],
                (SPC, H, C))),
        }
        in_maps.append(m)

    res = run_bass_kernel_spmd(nc, in_maps, list(range(N_CORES)))

    out1 = np.empty((B, C, H, W), np.float32)
    out2 = np.empty((B, C, H, W), np.float32)
    for core, pair in enumerate(pairs):
        r = res.results[core]
        o1 = r["o1"].reshape(SPC, H, C, W).transpose(0, 2, 1, 3)
        o2 = r["o2"].reshape(SPC, H, C, W).transpose(0, 2, 1, 3)
        for i, s in enumerate(pair):
            out1[s] = o1[i]
            out2[s] = o2[i]
    return out1, out2


# revision 2
# speedup vs baseline: 1.8392x; 1.8392x over previous
"""Trainium2 Bass kernel for nn_DynamicKernelSelection (moe_routing).

Strategy
--------
Host (cheap, O(B*C)):
  * Gating in float64 (argmax margins are far above fp32 noise).
  * Samples are paired by identical (expert1, expert2); at most one
    leftover pair can mix experts (pigeonhole) -- the device then runs that
    pair with slot-0's experts and the slot-1 sample is recomputed on the
    host (fp64, tiny fraction of total work).
  * Depthwise conv -> banded Toeplitz lhsT per (channel, kernel column):
    T[h', h] = W[c, dh, dw] at h' = h + (dh-k//2)*dil.  The H-taps become a
    single fp32 128x128 matmul per kernel column; W-shifts are realized by
    accumulating the k_w matmuls into PSUM at shifted column ranges.

Device (SPMD, 8 cores):
  * Work unit = (pair, channel): both samples of a pair are interleaved in
    the free dim as (w, s) so one N=256 matmul covers the pair.
  * Channels are split 128/8: every core runs 16 channels of EVERY pair,
    so per-pair native kernel sizes (3/5 and 7/9/11) give a uniform
    instruction stream across cores AND perfect load balance.
  * Per unit: k1 matmuls -> PSUM -> bias-add evac (VectorE) -> out1 tile
    (DMAed out, reused as stage-2 rhs) -> k2 matmuls -> PSUM -> evac ->
    out2.  All DMAs are [128-partition x contiguous] transfers.
"""

import numpy as np

B, C, H, W = 16, 128, 128, 128
N_CORES = 8
CPC = C // N_CORES           # channels per core per pair (16)
NPAIR = B // 2               # 8 pairs
DIL1, DIL2 = 1, 3
K1S = {0: 3, 1: 5}           # stage-1 expert -> kernel size
K2S = {0: 7, 1: 9, 2: 11}

_PROGS = {}                  # signature -> compiled program


# --------------------------------------------------------------- host math
def _gating(x, aw1, ab1, aw2, ab2):
    pooled = x.astype(np.float64).mean(axis=(2, 3))
    l1 = pooled @ aw1.astype(np.float64).T + ab1.astype(np.float64)
    l2 = pooled @ aw2.astype(np.float64).T + ab2.astype(np.float64)
    return l1.argmax(axis=1), l2.argmax(axis=1)


def _band(wk, dil):
    """wk: [C, k, k] fp32 -> banded lhsT stack [C, H, k*H] fp32."""
    k = wk.shape[-1]
    t = np.zeros((C, H, k, H), np.float32)
    tv = t.transpose(1, 3, 0, 2)  # [h', h, C, dw] view
    c0 = k // 2
    for dh in range(k):
        d = (dh - c0) * dil
        h = np.arange(max(0, -d), H - max(0, d))
        tv[h + d, h] = wk[:, dh, :]
    return np.ascontiguousarray(t.reshape(C, H, k * H))


def _host_conv(x, wk, b, dil):
    """x [C,H,W] fp64, wk [C,k,k], b [C]: same-padded depthwise conv."""
    k = wk.shape[-1]
    c0 = k // 2
    out = np.zeros_like(x)
    for dh in range(k):
        for dw in range(k):
            dh_, dw_ = (dh - c0) * dil, (dw - c0) * dil
            hs = slice(max(0, -dh_), H - max(0, dh_))
            ws = slice(max(0, -dw_), W - max(0, dw_))
            hs2 = slice(max(0, dh_), H - max(0, -dh_))
            ws2 = slice(max(0, dw_), W - max(0, -dw_))
            out[:, hs, ws] += wk[:, dh, dw][:, None, None] * x[:, hs2, ws2]
    return out + b[:, None, None]


def _pair_samples(idx1, idx2):
    """Pair samples by (e1, e2); leftovers paired preferring same e1.
    Returns pairs [(a, b)] and fixes {sample: 'stage2' | 'both'}."""
    from collections import defaultdict
    groups = defaultdict(list)
    for s in range(B):
        groups[(int(idx1[s]), int(idx2[s]))].append(s)
    pairs, singles = [], []
    for key in sorted(groups):
        lst = groups[key]
        while len(lst) >= 2:
            pairs.append((lst.pop(0), lst.pop(0)))
        if lst:
            singles.append(lst[0])
    # pair leftovers: prefer same e1 (then only stage-2 needs a host fix)
    fixes = {}
    while singles:
        a = singles.pop(0)
        bi = next((i for i, s in enumerate(singles)
                   if idx1[s] == idx1[a]), 0)
        b = singles.pop(bi)
        pairs.append((a, b))
        fixes[b] = "stage2" if idx1[b] == idx1[a] else "both"
    return pairs, fixes


# ------------------------------------------------------------ device program
def _build_program(sig):
    """sig: tuple of (k1, k2) per pair."""
    import concourse.tile as tile
    from concourse import bacc, mybir

    dt = mybir.dt.float32
    add = mybir.AluOpType.add
    nc = bacc.Bacc("TRN2", target_bir_lowering=False, debug=False,
                   enable_asserts=False, num_devices=N_CORES)

    xs_d, t1_d, t2_d, b1_d, b2_d, o1_d, o2_d = [], [], [], [], [], [], []
    for p, (k1, k2) in enumerate(sig):
        xs_d.append(nc.dram_tensor(f"x_{p}", [CPC, H, 2 * W], dt,
                                   kind="ExternalInput").ap())
        t1_d.append(nc.dram_tensor(f"t1_{p}", [CPC, H, k1 * H], dt,
                                   kind="ExternalInput").ap())
        t2_d.append(nc.dram_tensor(f"t2_{p}", [CPC, H, k2 * H], dt,
                                   kind="ExternalInput").ap())
        b1_d.append(nc.dram_tensor(f"b1_{p}", [H, CPC], dt,
                                   kind="ExternalInput").ap())
        b2_d.append(nc.dram_tensor(f"b2_{p}", [H, CPC], dt,
                                   kind="ExternalInput").ap())
        o1_d.append(nc.dram_tensor(f"o1_{p}", [CPC, H, 2 * W], dt,
                                   kind="ExternalOutput").ap())
        o2_d.append(nc.dram_tensor(f"o2_{p}", [CPC, H, 2 * W], dt,
                                   kind="ExternalOutput").ap())

    def conv_mms(psum, tt, src, k, dil):
        c0 = k // 2
        order = [c0] + [dw for dw in range(k) if dw != c0]
        for j, dw in enumerate(order):
            d = (dw - c0) * dil
            a = max(0, -d)
            ln = W - abs(d)
            nc.tensor.matmul(
                out=psum[:, 2 * a:2 * (a + ln)],
                lhsT=tt[:, dw * H:(dw + 1) * H],
                rhs=src[:, 2 * (a + d):2 * (a + d + ln)],
                start=(j == 0), stop=(j == len(order) - 1),
                skip_group_check=True)

    with tile.TileContext(nc) as tc:
        with (tc.tile_pool(name="xp", bufs=4) as xp,
              tc.tile_pool(name="o1p", bufs=3) as o1p,
              tc.tile_pool(name="o2p", bufs=3) as o2p,
              tc.tile_pool(name="t1p", bufs=3) as t1p,
              tc.tile_pool(name="t2p", bufs=3) as t2p,
              tc.tile_pool(name="bp", bufs=2) as bp,
              tc.tile_pool(name="ps", bufs=6, space="PSUM") as ps):
            for p, (k1, k2) in enumerate(sig):
                b1t = bp.tile([128, CPC], dt, tag="b1")
                nc.sync.dma_start(out=b1t[:], in_=b1_d[p])
                b2t = bp.tile([128, CPC], dt, tag="b2")
                nc.sync.dma_start(out=b2t[:], in_=b2_d[p])
                for u in range(CPC):
                    xc = xp.tile([128, 2 * W], dt, tag="x")
                    nc.sync.dma_start(out=xc[:], in_=xs_d[p][u])
                    t1t = t1p.tile([128, k1 * H], dt, tag="t1")
                    nc.sync.dma_start(out=t1t[:], in_=t1_d[p][u])
                    p1 = ps.tile([128, 2 * W], dt, tag="ps")
                    conv_mms(p1, t1t, xc, k1, DIL1)
                    o1c = o1p.tile([128, 2 * W], dt, tag="o1")
                    nc.vector.tensor_scalar(out=o1c[:], in0=p1[:],
                                            scalar1=b1t[:, u:u + 1],
                                            scalar2=None, op0=add)
                    nc.sync.dma_start(out=o1_d[p][u], in_=o1c[:])
                    t2t = t2p.tile([128, k2 * H], dt, tag="t2")
                    nc.sync.dma_start(out=t2t[:], in_=t2_d[p][u])
                    p2 = ps.tile([128, 2 * W], dt, tag="ps")
                    conv_mms(p2, t2t, o1c, k2, DIL2)
                    o2c = o2p.tile([128, 2 * W], dt, tag="o2")
                    nc.vector.tensor_scalar(out=o2c[:], in0=p2[:],
                                            scalar1=b2t[:, u:u + 1],
                                            scalar2=None, op0=add)
                    nc.sync.dma_start(out=o2_d[p][u], in_=o2c[:])
    nc.compile()
    return nc


# ------------------------------------------------------------------- driver
def kernel(x, aw1, ab1, aw2, ab2, w1_3, b1_3, w1_5, b1_5,
           w2_7, b2_7, w2_9, b2_9, w2_11, b2_11):
    from concourse.bass_utils import run_bass_kernel_spmd

    x = np.ascontiguousarray(np.asarray(x, dtype=np.float32))
    assert x.shape == (B, C, H, W)

    idx1, idx2 = _gating(np.asarray(x), np.asarray(aw1), np.asarray(ab1),
                         np.asarray(aw2), np.asarray(ab2))
    pairs, fixes = _pair_samples(idx1, idx2)

    w1e = [np.ascontiguousarray(np.asarray(w, np.float32)[:, 0])
           for w in (w1_3, w1_5)]
    w2e = [np.ascontiguousarray(np.asarray(w, np.float32)[:, 0])
           for w in (w2_7, w2_9, w2_11)]
    b1e = [np.asarray(b, np.float32) for b in (b1_3, b1_5)]
    b2e = [np.asarray(b, np.float32) for b in (b2_7, b2_9, b2_11)]

    # per-pair experts = slot-0's selection
    pe1 = [int(idx1[a]) for a, _ in pairs]
    pe2 = [int(idx2[a]) for a, _ in pairs]
    sig = tuple((K1S[e1], K2S[e2]) for e1, e2 in zip(pe1, pe2))

    if sig not in _PROGS:
        _PROGS[sig] = _build_program(sig)
    nc = _PROGS[sig]

    # banded lhsT per distinct expert actually used
    t1b = {e: _band(w1e[e], DIL1) for e in set(pe1)}
    t2b = {e: _band(w2e[e], DIL2) for e in set(pe2)}

    # per-pair interleaved x: [C, H, W, 2] -> [C, H, 2W]
    xpair = []
    for a, b in pairs:
        xi = np.stack([x[a], x[b]], axis=-1).reshape(C, H, 2 * W)
        xpair.append(xi)

    in_maps = []
    for core in range(N_CORES):
        cs = slice(core * CPC, (core + 1) * CPC)
        m = {}
        for p, (e1, e2) in enumerate(zip(pe1, pe2)):
            m[f"x_{p}"] = xpair[p][cs]
            m[f"t1_{p}"] = t1b[e1][cs]
            m[f"t2_{p}"] = t2b[e2][cs]
            m[f"b1_{p}"] = np.ascontiguousarray(
                np.broadcast_to(b1e[e1][None, cs], (H, CPC)))
            m[f"b2_{p}"] = np.ascontiguousarray(
                np.broadcast_to(b2e[e2][None, cs], (H, CPC)))
        in_maps.append(m)

    res = run_bass_kernel_spmd(nc, in_maps, list(range(N_CORES)))

    out1 = np.empty((B, C, H, W), np.float32)
    out2 = np.empty((B, C, H, W), np.float32)
    for core in range(N_CORES):
        cs = slice(core * CPC, (core + 1) * CPC)
        r = res.results[core]
        for p, (a, b) in enumerate(pairs):
            o1 = r[f"o1_{p}"].reshape(CPC, H, W, 2)
            o2 = r[f"o2_{p}"].reshape(CPC, H, W, 2)
            out1[a, cs], out1[b, cs] = o1[..., 0], o1[..., 1]
            out2[a, cs], out2[b, cs] = o2[..., 0], o2[..., 1]

    # host fix-up for mixed pairs (at most 2 samples)
    for s, kind in fixes.items():
        e1, e2 = int(idx1[s]), int(idx2[s])
        if kind == "both":
            o1 = _host_conv(x[s].astype(np.float64), w1e[e1].astype(np.float64),
                            b1e[e1].astype(np.float64), DIL1)
            out1[s] = o1.astype(np.float32)
        else:
            o1 = out1[s].astype(np.float64)
        out2[s] = _host_conv(o1, w2e[e2].astype(np.float64),
                             b2e[e2].astype(np.float64), DIL2).astype(np.float32)
    return out1, out2


# revision 3
# speedup vs baseline: 1.8419x; 1.0015x over previous
"""Trainium2 Bass kernel for nn_DynamicKernelSelection (moe_routing).

Strategy
--------
Host (cheap, O(B*C)):
  * Gating in float64 (argmax margins are far above fp32 noise).
  * Samples are paired by identical (expert1, expert2); at most one
    leftover pair can mix experts (pigeonhole) -- the device then runs that
    pair with slot-0's experts and the slot-1 sample is recomputed on the
    host (fp64, tiny fraction of total work).
  * Depthwise conv -> banded Toeplitz lhsT per (channel, kernel column):
    T[h', h] = W[c, dh, dw] at h' = h + (dh-k//2)*dil.  The H-taps become a
    single fp32 128x128 matmul per kernel column; W-shifts are realized by
    accumulating the k_w matmuls into PSUM at shifted column ranges.

Device (SPMD, 8 cores):
  * Work unit = (pair, channel): both samples of a pair are interleaved in
    the free dim as (w, s) so one N=256 matmul covers the pair.
  * Channels are split 128/8: every core runs 16 channels of EVERY pair,
    so per-pair native kernel sizes (3/5 and 7/9/11) give a uniform
    instruction stream across cores AND perfect load balance.
  * Per unit: k1 matmuls -> PSUM -> bias-add evac (VectorE) -> out1 tile
    (DMAed out, reused as stage-2 rhs) -> k2 matmuls -> PSUM -> evac ->
    out2.  All DMAs are [128-partition x contiguous] transfers.
"""

import numpy as np

B, C, H, W = 16, 128, 128, 128
N_CORES = 8
CPC = C // N_CORES           # channels per core per pair (16)
NPAIR = B // 2               # 8 pairs
DIL1, DIL2 = 1, 3
K1S = {0: 3, 1: 5}           # stage-1 expert -> kernel size
K2S = {0: 7, 1: 9, 2: 11}

_PROGS = {}                  # signature -> compiled program


# --------------------------------------------------------------- host math
def _gating(x, aw1, ab1, aw2, ab2):
    pooled = x.astype(np.float64).mean(axis=(2, 3))
    l1 = pooled @ aw1.astype(np.float64).T + ab1.astype(np.float64)
    l2 = pooled @ aw2.astype(np.float64).T + ab2.astype(np.float64)
    return l1.argmax(axis=1), l2.argmax(axis=1)


def _band(wk, dil):
    """wk: [C, k, k] fp32 -> banded lhsT stack [C, H, k*H] fp32."""
    k = wk.shape[-1]
    t = np.zeros((C, H, k, H), np.float32)
    tv = t.transpose(1, 3, 0, 2)  # [h', h, C, dw] view
    c0 = k // 2
    for dh in range(k):
        d = (dh - c0) * dil
        h = np.arange(max(0, -d), H - max(0, d))
        tv[h + d, h] = wk[:, dh, :]
    return np.ascontiguousarray(t.reshape(C, H, k * H))


def _host_conv(x, wk, b, dil):
    """x [C,H,W] fp64, wk [C,k,k], b [C]: same-padded depthwise conv."""
    k = wk.shape[-1]
    c0 = k // 2
    out = np.zeros_like(x)
    for dh in range(k):
        for dw in range(k):
            dh_, dw_ = (dh - c0) * dil, (dw - c0) * dil
            hs = slice(max(0, -dh_), H - max(0, dh_))
            ws = slice(max(0, -dw_), W - max(0, dw_))
            hs2 = slice(max(0, dh_), H - max(0, -dh_))
            ws2 = slice(max(0, dw_), W - max(0, -dw_))
            out[:, hs, ws] += wk[:, dh, dw][:, None, None] * x[:, hs2, ws2]
    return out + b[:, None, None]


def _pair_samples(idx1, idx2):
    """Pair samples by (e1, e2); leftovers paired preferring same e1.
    Returns pairs [(a, b)] and fixes {sample: 'stage2' | 'both'}."""
    from collections import defaultdict
    groups = defaultdict(list)
    for s in range(B):
        groups[(int(idx1[s]), int(idx2[s]))].append(s)
    pairs, singles = [], []
    for key in sorted(groups):
        lst = groups[key]
        while len(lst) >= 2:
            pairs.append((lst.pop(0), lst.pop(0)))
        if lst:
            singles.append(lst[0])
    # pair leftovers: prefer same e1 (then only stage-2 needs a host fix)
    fixes = {}
    while singles:
        a = singles.pop(0)
        bi = next((i for i, s in enumerate(singles)
                   if idx1[s] == idx1[a]), 0)
        b = singles.pop(bi)
        pairs.append((a, b))
        fixes[b] = "stage2" if idx1[b] == idx1[a] else "both"
    return pairs, fixes


# ------------------------------------------------------------ device program
def _build_program(sig):
    """sig: tuple of (k1, k2) per pair."""
    import concourse.tile as tile
    from concourse import bacc, mybir

    dt = mybir.dt.float32
    add = mybir.AluOpType.add
    nc = bacc.Bacc("TRN2", target_bir_lowering=False, debug=False,
                   enable_asserts=False, num_devices=N_CORES)

    xs_d, t1_d, t2_d, b1_d, b2_d, o1_d, o2_d = [], [], [], [], [], [], []
    for p, (k1, k2) in enumerate(sig):
        xs_d.append(nc.dram_tensor(f"x_{p}", [CPC, H, 2 * W], dt,
                                   kind="ExternalInput").ap())
        t1_d.append(nc.dram_tensor(f"t1_{p}", [CPC, H, k1 * H], dt,
                                   kind="ExternalInput").ap())
        t2_d.append(nc.dram_tensor(f"t2_{p}", [CPC, H, k2 * H], dt,
                                   kind="ExternalInput").ap())
        b1_d.append(nc.dram_tensor(f"b1_{p}", [H, CPC], dt,
                                   kind="ExternalInput").ap())
        b2_d.append(nc.dram_tensor(f"b2_{p}", [H, CPC], dt,
                                   kind="ExternalInput").ap())
        o1_d.append(nc.dram_tensor(f"o1_{p}", [CPC, H, 2 * W], dt,
                                   kind="ExternalOutput").ap())
        o2_d.append(nc.dram_tensor(f"o2_{p}", [CPC, H, 2 * W], dt,
                                   kind="ExternalOutput").ap())

    def conv_mms(psum, tt, src, k, dil):
        c0 = k // 2
        order = [c0] + [dw for dw in range(k) if dw != c0]
        for j, dw in enumerate(order):
            d = (dw - c0) * dil
            a = max(0, -d)
            ln = W - abs(d)
            nc.tensor.matmul(
                out=psum[:, 2 * a:2 * (a + ln)],
                lhsT=tt[:, dw * H:(dw + 1) * H],
                rhs=src[:, 2 * (a + d):2 * (a + d + ln)],
                start=(j == 0), stop=(j == len(order) - 1),
                skip_group_check=True)

    with tile.TileContext(nc) as tc:
        with (tc.tile_pool(name="xp", bufs=6) as xp,
              tc.tile_pool(name="o1p", bufs=4) as o1p,
              tc.tile_pool(name="o2p", bufs=4) as o2p,
              tc.tile_pool(name="t1p", bufs=4) as t1p,
              tc.tile_pool(name="t2p", bufs=4) as t2p,
              tc.tile_pool(name="bp", bufs=2) as bp,
              tc.tile_pool(name="ps", bufs=8, space="PSUM") as ps):
            for p, (k1, k2) in enumerate(sig):
                b1t = bp.tile([128, CPC], dt, tag="b1")
                nc.sync.dma_start(out=b1t[:], in_=b1_d[p])
                b2t = bp.tile([128, CPC], dt, tag="b2")
                nc.sync.dma_start(out=b2t[:], in_=b2_d[p])
                for u in range(CPC):
                    xc = xp.tile([128, 2 * W], dt, tag="x")
                    nc.sync.dma_start(out=xc[:], in_=xs_d[p][u])
                    t1t = t1p.tile([128, k1 * H], dt, tag="t1")
                    nc.sync.dma_start(out=t1t[:], in_=t1_d[p][u])
                    p1 = ps.tile([128, 2 * W], dt, tag="ps")
                    conv_mms(p1, t1t, xc, k1, DIL1)
                    o1c = o1p.tile([128, 2 * W], dt, tag="o1")
                    nc.vector.tensor_scalar(out=o1c[:], in0=p1[:],
                                            scalar1=b1t[:, u:u + 1],
                                            scalar2=None, op0=add)
                    nc.sync.dma_start(out=o1_d[p][u], in_=o1c[:])
                    t2t = t2p.tile([128, k2 * H], dt, tag="t2")
                    nc.sync.dma_start(out=t2t[:], in_=t2_d[p][u])
                    p2 = ps.tile([128, 2 * W], dt, tag="ps")
                    conv_mms(p2, t2t, o1c, k2, DIL2)
                    o2c = o2p.tile([128, 2 * W], dt, tag="o2")
                    nc.vector.tensor_scalar(out=o2c[:], in0=p2[:],
                                            scalar1=b2t[:, u:u + 1],
                                            scalar2=None, op0=add)
                    nc.sync.dma_start(out=o2_d[p][u], in_=o2c[:])
    nc.compile()
    return nc


# ------------------------------------------------------------------- driver
def kernel(x, aw1, ab1, aw2, ab2, w1_3, b1_3, w1_5, b1_5,
           w2_7, b2_7, w2_9, b2_9, w2_11, b2_11):
    from concourse.bass_utils import run_bass_kernel_spmd

    x = np.ascontiguousarray(np.asarray(x, dtype=np.float32))
    assert x.shape == (B, C, H, W)

    idx1, idx2 = _gating(np.asarray(x), np.asarray(aw1), np.asarray(ab1),
                         np.asarray(aw2), np.asarray(ab2))
    pairs, fixes = _pair_samples(idx1, idx2)

    w1e = [np.ascontiguousarray(np.asarray(w, np.float32)[:, 0])
           for w in (w1_3, w1_5)]
    w2e = [np.ascontiguousarray(np.asarray(w, np.float32)[:, 0])
           for w in (w2_7, w2_9, w2_11)]
    b1e = [np.asarray(b, np.float32) for b in (b1_3, b1_5)]
    b2e = [np.asarray(b, np.float32) for b in (b2_7, b2_9, b2_11)]

    # per-pair experts = slot-0's selection
    pe1 = [int(idx1[a]) for a, _ in pairs]
    pe2 = [int(idx2[a]) for a, _ in pairs]
    sig = tuple((K1S[e1], K2S[e2]) for e1, e2 in zip(pe1, pe2))

    if sig not in _PROGS:
        _PROGS[sig] = _build_program(sig)
    nc = _PROGS[sig]

    # banded lhsT per distinct expert actually used
    t1b = {e: _band(w1e[e], DIL1) for e in set(pe1)}
    t2b = {e: _band(w2e[e], DIL2) for e in set(pe2)}

    # per-pair interleaved x: [C, H, W, 2] -> [C, H, 2W]
    xpair = []
    for a, b in pairs:
        xi = np.stack([x[a], x[b]], axis=-1).reshape(C, H, 2 * W)
        xpair.append(xi)

    in_maps = []
    for core in range(N_CORES):
        cs = slice(core * CPC, (core + 1) * CPC)
        m = {}
        for p, (e1, e2) in enumerate(zip(pe1, pe2)):
            m[f"x_{p}"] = xpair[p][cs]
            m[f"t1_{p}"] = t1b[e1][cs]
            m[f"t2_{p}"] = t2b[e2][cs]
            m[f"b1_{p}"] = np.ascontiguousarray(
                np.broadcast_to(b1e[e1][None, cs], (H, CPC)))
            m[f"b2_{p}"] = np.ascontiguousarray(
                np.broadcast_to(b2e[e2][None, cs], (H, CPC)))
        in_maps.append(m)

    res = run_bass_kernel_spmd(nc, in_maps, list(range(N_CORES)))

    out1 = np.empty((B, C, H, W), np.float32)
    out2 = np.empty((B, C, H, W), np.float32)
    for core in range(N_CORES):
        cs = slice(core * CPC, (core + 1) * CPC)
        r = res.results[core]
        for p, (a, b) in enumerate(pairs):
            o1 = r[f"o1_{p}"].reshape(CPC, H, W, 2)
            o2 = r[f"o2_{p}"].reshape(CPC, H, W, 2)
            out1[a, cs], out1[b, cs] = o1[..., 0], o1[..., 1]
            out2[a, cs], out2[b, cs] = o2[..., 0], o2[..., 1]

    # host fix-up for mixed pairs (at most 2 samples)
    for s, kind in fixes.items():
        e1, e2 = int(idx1[s]), int(idx2[s])
        if kind == "both":
            o1 = _host_conv(x[s].astype(np.float64), w1e[e1].astype(np.float64),
                            b1e[e1].astype(np.float64), DIL1)
            out1[s] = o1.astype(np.float32)
        else:
            o1 = out1[s].astype(np.float64)
        out2[s] = _host_conv(o1, w2e[e2].astype(np.float64),
                             b2e[e2].astype(np.float64), DIL2).astype(np.float32)
    return out1, out2


# revision 4
# speedup vs baseline: 2.1414x; 1.1626x over previous
"""Trainium2 Bass kernel for nn_DynamicKernelSelection (moe_routing).

Strategy
--------
Host (cheap, O(B*C)):
  * Gating in float64 (argmax margins are far above fp32 noise).
  * Samples are paired by identical (expert1, expert2); at most one
    leftover pair can mix experts (pigeonhole) -- the device then runs that
    pair with slot-0's experts and the slot-1 sample is recomputed on the
    host (fp64, tiny fraction of total work).
  * Depthwise conv -> banded Toeplitz lhsT per (channel, kernel column):
    T[h', h] = W[c, dh, dw] at h' = h + (dh-k//2)*dil.  The H-taps become a
    single fp32 128x128 matmul per kernel column; W-shifts are realized by
    accumulating the k_w matmuls into PSUM at shifted column ranges.

Device (SPMD, 8 cores):
  * Work unit = (pair, channel): both samples of a pair are interleaved in
    the free dim as (w, s) so one N=256 matmul covers the pair.
  * Channels are split 128/8: every core runs 16 channels of EVERY pair,
    so per-pair native kernel sizes (3/5 and 7/9/11) give a uniform
    instruction stream across cores AND perfect load balance.
  * Per unit: k1 matmuls -> PSUM -> bias-add evac (VectorE) -> out1 tile
    (DMAed out, reused as stage-2 rhs) -> k2 matmuls -> PSUM -> evac ->
    out2.  All DMAs are [128-partition x contiguous] transfers.
"""

import numpy as np

B, C, H, W = 16, 128, 128, 128
N_CORES = 8
CPC = C // N_CORES           # channels per core per pair (16)
NPAIR = B // 2               # 8 pairs
DIL1, DIL2 = 1, 3
K1S = {0: 3, 1: 5}           # stage-1 expert -> kernel size
K2S = {0: 7, 1: 9, 2: 11}

_PROGS = {}                  # signature -> compiled program


# --------------------------------------------------------------- host math
def _gating(x, aw1, ab1, aw2, ab2):
    pooled = x.astype(np.float64).mean(axis=(2, 3))
    l1 = pooled @ aw1.astype(np.float64).T + ab1.astype(np.float64)
    l2 = pooled @ aw2.astype(np.float64).T + ab2.astype(np.float64)
    return l1.argmax(axis=1), l2.argmax(axis=1)


def _band(wk, dil):
    """wk: [C, k, k] fp32 -> banded lhsT stack [C, H, k*H] fp32."""
    k = wk.shape[-1]
    t = np.zeros((C, H, k, H), np.float32)
    tv = t.transpose(1, 3, 0, 2)  # [h', h, C, dw] view
    c0 = k // 2
    for dh in range(k):
        d = (dh - c0) * dil
        h = np.arange(max(0, -d), H - max(0, d))
        tv[h + d, h] = wk[:, dh, :]
    return np.ascontiguousarray(t.reshape(C, H, k * H))


def _host_conv(x, wk, b, dil):
    """x [C,H,W] fp64, wk [C,k,k], b [C]: same-padded depthwise conv."""
    k = wk.shape[-1]
    c0 = k // 2
    out = np.zeros_like(x)
    for dh in range(k):
        for dw in range(k):
            dh_, dw_ = (dh - c0) * dil, (dw - c0) * dil
            hs = slice(max(0, -dh_), H - max(0, dh_))
            ws = slice(max(0, -dw_), W - max(0, dw_))
            hs2 = slice(max(0, dh_), H - max(0, -dh_))
            ws2 = slice(max(0, dw_), W - max(0, -dw_))
            out[:, hs, ws] += wk[:, dh, dw][:, None, None] * x[:, hs2, ws2]
    return out + b[:, None, None]


def _pair_samples(idx1, idx2):
    """Pair samples by (e1, e2); leftovers paired preferring same e1.
    Returns pairs [(a, b)] and fixes {sample: 'stage2' | 'both'}."""
    from collections import defaultdict
    groups = defaultdict(list)
    for s in range(B):
        groups[(int(idx1[s]), int(idx2[s]))].append(s)
    pairs, singles = [], []
    for key in sorted(groups):
        lst = groups[key]
        while len(lst) >= 2:
            pairs.append((lst.pop(0), lst.pop(0)))
        if lst:
            singles.append(lst[0])
    # pair leftovers: prefer same e1 (then only stage-2 needs a host fix)
    fixes = {}
    while singles:
        a = singles.pop(0)
        bi = next((i for i, s in enumerate(singles)
                   if idx1[s] == idx1[a]), 0)
        b = singles.pop(bi)
        pairs.append((a, b))
        fixes[b] = "stage2" if idx1[b] == idx1[a] else "both"
    return pairs, fixes


# ------------------------------------------------------------ device program
def _build_program(sig):
    """sig: tuple of (k1, k2) per pair."""
    import concourse.tile as tile
    from concourse import bacc, mybir

    dt = mybir.dt.float32
    add = mybir.AluOpType.add
    nc = bacc.Bacc("TRN2", target_bir_lowering=False, debug=False,
                   enable_asserts=False, num_devices=N_CORES)

    xs_d, t1_d, t2_d, b1_d, b2_d, o1_d, o2_d = [], [], [], [], [], [], []
    for p, (k1, k2) in enumerate(sig):
        xs_d.append(nc.dram_tensor(f"x_{p}", [CPC, H, 2 * W], dt,
                                   kind="ExternalInput").ap())
        t1_d.append(nc.dram_tensor(f"t1_{p}", [CPC, H, k1 * H], dt,
                                   kind="ExternalInput").ap())
        t2_d.append(nc.dram_tensor(f"t2_{p}", [CPC, H, k2 * H], dt,
                                   kind="ExternalInput").ap())
        b1_d.append(nc.dram_tensor(f"b1_{p}", [H, CPC], dt,
                                   kind="ExternalInput").ap())
        b2_d.append(nc.dram_tensor(f"b2_{p}", [H, CPC], dt,
                                   kind="ExternalInput").ap())
        o1_d.append(nc.dram_tensor(f"o1_{p}", [CPC, H, 2 * W], dt,
                                   kind="ExternalOutput").ap())
        o2_d.append(nc.dram_tensor(f"o2_{p}", [CPC, H, 2 * W], dt,
                                   kind="ExternalOutput").ap())

    def conv_mms(psum, tt, src, k, dil):
        c0 = k // 2
        order = [c0] + [dw for dw in range(k) if dw != c0]
        for j, dw in enumerate(order):
            d = (dw - c0) * dil
            a = max(0, -d)
            ln = W - abs(d)
            nc.tensor.matmul(
                out=psum[:, 2 * a:2 * (a + ln)],
                lhsT=tt[:, dw * H:(dw + 1) * H],
                rhs=src[:, 2 * (a + d):2 * (a + d + ln)],
                start=(j == 0), stop=(j == len(order) - 1),
                skip_group_check=True)

    with tile.TileContext(nc) as tc:
        with (tc.tile_pool(name="xp", bufs=6) as xp,
              tc.tile_pool(name="o1p", bufs=4) as o1p,
              tc.tile_pool(name="o2p", bufs=4) as o2p,
              tc.tile_pool(name="t1p", bufs=4) as t1p,
              tc.tile_pool(name="t2p", bufs=4) as t2p,
              tc.tile_pool(name="bp", bufs=2) as bp,
              tc.tile_pool(name="ps", bufs=8, space="PSUM") as ps):
            # software-pipelined by one unit: emit stage2(prev) after
            # stage1(cur) so PE never waits on the DVE bias-evac.
            pending = None

            def emit_stage2(st):
                p, u, k2, o1c, b2t = st
                t2t = t2p.tile([128, k2 * H], dt, tag="t2")
                nc.sync.dma_start(out=t2t[:], in_=t2_d[p][u])
                p2 = ps.tile([128, 2 * W], dt, tag="ps")
                conv_mms(p2, t2t, o1c, k2, DIL2)
                o2c = o2p.tile([128, 2 * W], dt, tag="o2")
                nc.vector.tensor_scalar(out=o2c[:], in0=p2[:],
                                        scalar1=b2t[:, u:u + 1],
                                        scalar2=None, op0=add)
                nc.sync.dma_start(out=o2_d[p][u], in_=o2c[:])

            for p, (k1, k2) in enumerate(sig):
                b1t = bp.tile([128, CPC], dt, tag="b1")
                nc.sync.dma_start(out=b1t[:], in_=b1_d[p])
                b2t = bp.tile([128, CPC], dt, tag="b2")
                nc.sync.dma_start(out=b2t[:], in_=b2_d[p])
                for u in range(CPC):
                    xc = xp.tile([128, 2 * W], dt, tag="x")
                    nc.sync.dma_start(out=xc[:], in_=xs_d[p][u])
                    t1t = t1p.tile([128, k1 * H], dt, tag="t1")
                    nc.sync.dma_start(out=t1t[:], in_=t1_d[p][u])
                    p1 = ps.tile([128, 2 * W], dt, tag="ps")
                    conv_mms(p1, t1t, xc, k1, DIL1)
                    o1c = o1p.tile([128, 2 * W], dt, tag="o1")
                    nc.vector.tensor_scalar(out=o1c[:], in0=p1[:],
                                            scalar1=b1t[:, u:u + 1],
                                            scalar2=None, op0=add)
                    nc.sync.dma_start(out=o1_d[p][u], in_=o1c[:])
                    if pending is not None:
                        emit_stage2(pending)
                    pending = (p, u, k2, o1c, b2t)
            emit_stage2(pending)
    nc.compile()
    return nc


# ------------------------------------------------------------------- driver
def kernel(x, aw1, ab1, aw2, ab2, w1_3, b1_3, w1_5, b1_5,
           w2_7, b2_7, w2_9, b2_9, w2_11, b2_11):
    from concourse.bass_utils import run_bass_kernel_spmd

    x = np.ascontiguousarray(np.asarray(x, dtype=np.float32))
    assert x.shape == (B, C, H, W)

    idx1, idx2 = _gating(np.asarray(x), np.asarray(aw1), np.asarray(ab1),
                         np.asarray(aw2), np.asarray(ab2))
    pairs, fixes = _pair_samples(idx1, idx2)

    w1e = [np.ascontiguousarray(np.asarray(w, np.float32)[:, 0])
           for w in (w1_3, w1_5)]
    w2e = [np.ascontiguousarray(np.asarray(w, np.float32)[:, 0])
           for w in (w2_7, w2_9, w2_11)]
    b1e = [np.asarray(b, np.float32) for b in (b1_3, b1_5)]
    b2e = [np.asarray(b, np.float32) for b in (b2_7, b2_9, b2_11)]

    # per-pair experts = slot-0's selection
    pe1 = [int(idx1[a]) for a, _ in pairs]
    pe2 = [int(idx2[a]) for a, _ in pairs]
    sig = tuple((K1S[e1], K2S[e2]) for e1, e2 in zip(pe1, pe2))

    if sig not in _PROGS:
        _PROGS[sig] = _build_program(sig)
    nc = _PROGS[sig]

    # banded lhsT per distinct expert actually used
    t1b = {e: _band(w1e[e], DIL1) for e in set(pe1)}
    t2b = {e: _band(w2e[e], DIL2) for e in set(pe2)}

    # per-pair interleaved x: [C, H, W, 2] -> [C, H, 2W]
    xpair = []
    for a, b in pairs:
        xi = np.stack([x[a], x[b]], axis=-1).reshape(C, H, 2 * W)
        xpair.append(xi)

    in_maps = []
    for core in range(N_CORES):
        cs = slice(core * CPC, (core + 1) * CPC)
        m = {}
        for p, (e1, e2) in enumerate(zip(pe1, pe2)):
            m[f"x_{p}"] = xpair[p][cs]
            m[f"t1_{p}"] = t1b[e1][cs]
            m[f"t2_{p}"] = t2b[e2][cs]
            m[f"b1_{p}"] = np.ascontiguousarray(
                np.broadcast_to(b1e[e1][None, cs], (H, CPC)))
            m[f"b2_{p}"] = np.ascontiguousarray(
                np.broadcast_to(b2e[e2][None, cs], (H, CPC)))
        in_maps.append(m)

    res = run_bass_kernel_spmd(nc, in_maps, list(range(N_CORES)))

    out1 = np.empty((B, C, H, W), np.float32)
    out2 = np.empty((B, C, H, W), np.float32)
    for core in range(N_CORES):
        cs = slice(core * CPC, (core + 1) * CPC)
        r = res.results[core]
        for p, (a, b) in enumerate(pairs):
            o1 = r[f"o1_{p}"].reshape(CPC, H, W, 2)
            o2 = r[f"o2_{p}"].reshape(CPC, H, W, 2)
            out1[a, cs], out1[b, cs] = o1[..., 0], o1[..., 1]
            out2[a, cs], out2[b, cs] = o2[..., 0], o2[..., 1]

    # host fix-up for mixed pairs (at most 2 samples)
    for s, kind in fixes.items():
        e1, e2 = int(idx1[s]), int(idx2[s])
        if kind == "both":
            o1 = _host_conv(x[s].astype(np.float64), w1e[e1].astype(np.float64),
                            b1e[e1].astype(np.float64), DIL1)
            out1[s] = o1.astype(np.float32)
        else:
            o1 = out1[s].astype(np.float64)
        out2[s] = _host_conv(o1, w2e[e2].astype(np.float64),
                             b2e[e2].astype(np.float64), DIL2).astype(np.float32)
    return out1, out2
